# revision 28
# baseline (speedup 1.0000x reference)
"""Multi-head attention (B=2, T=2048, d_model=1024, H=16, hd=64) on 8 Trainium2
NeuronCores.

Sharding: 4 consecutive heads of one batch per core (core c -> batch c//4,
heads 4*(c%4)..+3). Each core: QKV projection slice, causal attention, partial
out-projection (its 256 rows of W_out); host sums 4 partials/batch + b_out.

v2 layout (everything stays transposed end-to-end; no on-chip transposes):
  qT/kT [hd, T]   <- lhsT=W chunk, rhs=xT (2 heads per 128 partitions)
  sT    [k, q]    <- two K=64 matmuls, one per head, row-paired on the PE
                     (tile_position rows 0/64) -> concurrent
  E     [k, q]    <- exp(sT/8) on ScalarE (bf16); diagonal blocks masked
                     post-exp with a 0/1 triangle; above-diagonal blocks never
                     computed (scores/exp/pv all narrowed to the causal band)
  pvT   [65, q]   <- lhsT=[v|1] chunk, rhs=E chunk: rows 0-63 unnormalized
                     aT, row 64 = softmax denominators (free)
  aT    [hd, q]   = pvT[0:64] * bcast(1/pvT[64]) (recip_approx_fast on the
                     psum row + stride-0 broadcast DMA + one DVE multiply)
  out  += aT.T @ W_out chunk  (psum -> DRAM DMA directly, fp32)

Emission is software-pipelined: head-pair 0's attention starts right after
its projections (pair-1 projections + v tiles ride along as fillers); pv lags
scores by one q-group so the PE never stalls on exp; out-projection for
q-group g is emitted inside head-pair 1's loop right after group g finishes.
"""

import math
import os
from contextlib import ExitStack
from dataclasses import dataclass

import numpy as np
import ml_dtypes

import concourse.bass as bass
import concourse.tile as tile
from concourse import bacc, mybir
from concourse import bass_utils

AF = mybir.ActivationFunctionType
ALU = mybir.AluOpType
DT = mybir.dt

N_CORES = 8


@dataclass(frozen=True)
class Cfg:
    T: int = 2048        # sequence length
    DM: int = 1024       # d_model
    HD: int = 64         # head dim
    NH: int = 4          # heads per core
    mode: str = "causal"
    mm: str = "bf16"
    dbg: bool = False

    @property
    def NHD(self):
        return self.NH * self.HD          # qkv slice width per core

    @property
    def KC(self):
        return self.DM // 128             # contraction chunks for projections

    @property
    def MC(self):
        return self.NHD // 128            # head-pair chunks

    @property
    def TC(self):
        return self.T // 128              # t chunks

    @property
    def QW(self):
        return 512                        # q group width

    @property
    def QG(self):
        return self.T // self.QW

    @property
    def QT(self):
        return self.QW // 128             # q tiles (128) per group

    @property
    def npmm(self):
        return ml_dtypes.bfloat16


def build_program(cfg: Cfg):
    c = cfg
    assert c.mode == "causal"
    assert c.DM == 1024 and c.T == 2048 and c.HD == 64 and c.NH == 4
    nc = bacc.Bacc("TRN2", target_bir_lowering=False, debug=False,
                   num_devices=N_CORES)
    f32 = DT.float32
    bf16 = DT.bfloat16

    # xT has an extra 128-row chunk: row 1024 = ones (for the v bias matmul)
    xT = nc.dram_tensor("xT", [c.DM + 128, c.T], bf16, kind="ExternalInput").ap()
    wq = nc.dram_tensor("wq", [c.DM, c.NHD], bf16, kind="ExternalInput").ap()
    wk = nc.dram_tensor("wk", [c.DM, c.NHD], bf16, kind="ExternalInput").ap()
    # wv has the bias as row 1024 (contracts against the ones row of xT)
    wv = nc.dram_tensor("wv", [c.DM + 128, c.NHD], bf16, kind="ExternalInput").ap()
    bq = nc.dram_tensor("bq", [128, c.MC], f32, kind="ExternalInput").ap()
    bk = nc.dram_tensor("bk", [128, c.MC], f32, kind="ExternalInput").ap()
    wo = nc.dram_tensor("wo", [c.NHD, c.DM], bf16, kind="ExternalInput").ap()
    out = nc.dram_tensor("out", [c.T, c.DM], bf16, kind="ExternalOutput").ap()
    dbg = {}
    if c.dbg:
        dbg["qT"] = nc.dram_tensor("dbg_qT", [128, c.MC, c.T], bf16, kind="ExternalOutput").ap()
        dbg["kT"] = nc.dram_tensor("dbg_kT", [128, c.MC, c.T], bf16, kind="ExternalOutput").ap()
        dbg["v"] = nc.dram_tensor("dbg_v", [128, c.TC, c.NH, 128], bf16, kind="ExternalOutput").ap()
        dbg["aT"] = nc.dram_tensor("dbg_aT", [128, c.MC, c.T], bf16, kind="ExternalOutput").ap()

    with tile.TileContext(nc) as tc, ExitStack() as ctx:
        _body(ctx, tc, c, xT, wq, wk, wv, bq, bk, wo, out, dbg)
    nc.compile()
    return nc, ["xT", "wq", "wk", "wv", "bq", "bk", "wo"]


def _body(ctx, tc, c: Cfg, xT, wq, wk, wv, bq, bk, wo, out, dbg=None):
    nc = tc.nc
    f32 = DT.float32
    bf16 = DT.bfloat16
    scale = 1.0 / math.sqrt(c.HD)
    HD1 = c.HD + 1

    const = ctx.enter_context(tc.tile_pool(name="const", bufs=1))
    big = ctx.enter_context(tc.tile_pool(name="big", bufs=1))
    epool = ctx.enter_context(tc.tile_pool(name="E", bufs=24))
    rfpool = ctx.enter_context(tc.tile_pool(name="rf", bufs=2))
    rbpool = ctx.enter_context(tc.tile_pool(name="rb", bufs=2))
    stg = ctx.enter_context(tc.tile_pool(name="stg", bufs=2))
    # PSUM: 2 x [128,1024] (4 banks) + 4 x [128,512] (4 banks)
    ps_mm = ctx.enter_context(tc.tile_pool(name="psmm", bufs=2, space="PSUM"))
    ps_pv = ctx.enter_context(tc.tile_pool(name="pspv", bufs=4, space="PSUM"))

    # ---- input DMAs: spread across engine queues so they run in parallel
    # (a dma_start occupies its issuing engine's queue for the transfer;
    # all engines are idle at kernel start)
    XC = c.KC + 1
    xT_sb = big.tile([128, XC, c.T], bf16, tag="xT")
    xTd = xT.rearrange("(c p) t -> p c t", p=128)
    TH = c.T // 2
    # stream xT in chunk-triples so the first projection matmuls can start
    # after ~800KB instead of after the whole 4.5MB
    wq_sb = big.tile([128, c.KC, c.NHD], bf16, tag="wq")
    nc.scalar.dma_start(out=wq_sb[:], in_=wq.rearrange("(c p) n -> p c n", p=128))
    nc.sync.dma_start(out=xT_sb[:, 0:3, 0:TH], in_=xTd[:, 0:3, 0:TH])
    wk_sb = big.tile([128, c.KC, c.NHD], bf16, tag="wk")
    nc.scalar.dma_start(out=wk_sb[:], in_=wk.rearrange("(c p) n -> p c n", p=128))
    nc.sync.dma_start(out=xT_sb[:, 3:6, 0:TH], in_=xTd[:, 3:6, 0:TH])
    wv_sb = big.tile([128, XC, c.NHD], bf16, tag="wv")
    nc.scalar.dma_start(out=wv_sb[:], in_=wv.rearrange("(c p) n -> p c n", p=128))
    nc.sync.dma_start(out=xT_sb[:, 6:XC, 0:TH], in_=xTd[:, 6:XC, 0:TH])

    def emit_xT_h1():
        nc.sync.dma_start(out=xT_sb[:, 0:3, TH:c.T], in_=xTd[:, 0:3, TH:c.T])
        nc.sync.dma_start(out=xT_sb[:, 3:6, TH:c.T], in_=xTd[:, 3:6, TH:c.T])
        nc.sync.dma_start(out=xT_sb[:, 6:XC, TH:c.T],
                          in_=xTd[:, 6:XC, TH:c.T])

    bq_sb = const.tile([128, c.MC], f32, tag="bq")
    nc.gpsimd.dma_start(out=bq_sb[:], in_=bq)
    bk_sb = const.tile([128, c.MC], f32, tag="bk")
    nc.gpsimd.dma_start(out=bk_sb[:], in_=bk)

    wo_sb = big.tile([128, c.MC, c.DM], bf16, tag="wo")
    nc.scalar.dma_start(out=wo_sb[:], in_=wo.rearrange("(c p) n -> p c n", p=128))

    ones64 = const.tile([1, 64], f32, tag="ones64")
    nc.vector.memset(ones64[:], 1.0)

    # 0/1 lower-causal triangle: tri01[k, q] = 1 if q >= k else 0
    tri01 = const.tile([128, 128], bf16, tag="tri")
    nc.gpsimd.memset(tri01[:], 1.0)
    nc.gpsimd.affine_select(
        out=tri01[:], in_=tri01[:],
        compare_op=ALU.is_ge, fill=0.0,
        base=0, channel_multiplier=-1, pattern=[[1, 128]],
    )

    # ---- projection targets ----
    qT_sb = big.tile([128, c.MC, c.T], bf16, tag="qT")
    kT_sb = big.tile([128, c.MC, c.T], bf16, tag="kT")
    v_sb = big.tile([128, c.TC, c.NH, 128], bf16, tag="v")
    nc.vector.memset(v_sb[:, :, :, 0:c.HD], 0.0)
    nc.vector.memset(v_sb[:, :, :, 0:1], 1.0)
    aT_sb = big.tile([128, c.MC, c.T], bf16, tag="aT")

    W2 = 512                               # q/k psum tile width (t cols)
    VG = 2                                 # t-chunks per v psum tile

    def emit_qk_tile(m, which, n):
        """One [128, 512] psum tile of the q or k projection."""
        w_sb, b_sb, dst = ((wq_sb, bq_sb, qT_sb) if which == "q"
                           else (wk_sb, bk_sb, kT_sb))
        ps = ps_mm.tile([128, 1024], f32, tag="mm")
        for kc in range(c.KC):
            nc.tensor.matmul(
                ps[:, 0:W2],
                lhsT=w_sb[:, kc, m * 128:(m + 1) * 128],
                rhs=xT_sb[:, kc, n * W2:(n + 1) * W2],
                start=(kc == 0), stop=(kc == c.KC - 1),
            )
        nc.vector.tensor_scalar_add(
            dst[:, m, n * W2:(n + 1) * W2], ps[:, 0:W2], b_sb[:, m:m + 1],
        )

    def emit_v_tile(tg):
        """VG t-chunks of v for all heads (+ bias via the ones row chunk)."""
        ps = ps_mm.tile([128, 1024], f32, tag="mm")
        for d in range(VG):
            t = tg * VG + d
            for kc in range(XC):
                nc.tensor.matmul(
                    ps[:, d * c.NHD:(d + 1) * c.NHD],
                    lhsT=xT_sb[:, kc, t * 128:(t + 1) * 128],
                    rhs=wv_sb[:, kc, :],
                    start=(kc == 0), stop=(kc == XC - 1),
                )
        for d in range(VG):
            t = tg * VG + d
            nc.vector.tensor_copy(
                v_sb[:, t, :, c.HD:128],
                ps[:, d * c.NHD:(d + 1) * c.NHD].rearrange(
                    "p (h d) -> p h d", d=c.HD),
            )

    # ---- attention ----
    # Per (hp, g): kc ranges over the causal band 0..4g+3. For band chunks
    # (kc >= 4g) only columns q >= kc*128 exist anywhere: scores, exp and pv
    # are all narrowed. One [128,1024] psum tile per kc packs both heads of
    # the pair side by side [h0 w | h1 w] -> one ACT covers both.
    def jstart(kc, g):
        return max(0, kc - c.QT * g)

    # Rate-matched feeder: between score rounds the emitter interleaves
    # small work items (pv matmuls, normalization, out-projection tiles,
    # projection fillers) so the PE stays busy while ScalarE drains the exp
    # backlog, without running further ahead than the psum FIFO allows.
    # "urgent" items (pv/norm/outproj) free psum/E-pool slots and pop before
    # "bulk" fillers. Costs are PE-time estimates in ns.
    feed_urgent = []
    feed_state = {"credit": 0.0}

    def feed(budget_ns):
        feed_state["credit"] = min(feed_state["credit"] + budget_ns, 4000.0)
        while feed_state["credit"] > 0 and feed_urgent:
            cost, fn = feed_urgent.pop(0)
            fn()
            feed_state["credit"] -= cost

    def force(cost_ns, fn):
        """Emit a prerequisite blob now (runs during ACT backlog; no credit
        charge, else pv items pile up into the flush tail)."""
        fn()

    def flush_feed():
        while feed_urgent:
            _, fn = feed_urgent.pop(0)
            fn()
        feed_state["credit"] = 0.0

    def attn_group(hp, g):
        """Scores + exp + streamed pv for (hp, g).

        Rounds process kc pairs: per round two [128,1024] psum tiles (one
        per head) pack [w(kc0) | w(kc1)] so one ACT covers two kc chunks.
        pv matmuls for the same group are appended to the feeder as their
        E tiles complete, so the PE always has fill between score rounds.
        """
        kmax = (g + 1) * c.QT
        boxes = {0: {}, 1: {}}

        def pv_mm(hl, kc, box):
            if kc == 0:
                psv_t = ps_pv.tile([128, 512], f32, tag="pv")
                box["psv"] = psv_t
            psv = box["psv"]
            et, cb = box["ets"][kc // 2]
            j = jstart(kc, g)
            nc.tensor.matmul(
                psv[:, j * 128:512],
                lhsT=v_sb[:, kc, 2 * hp + hl, :],
                rhs=et[:, cb[kc % 2]:cb[kc % 2] + 512 - 128 * j],
                start=(kc == 0), stop=(kc == kmax - 1),
            )

        for kp in range(kmax // 2):
            kcs = (2 * kp, 2 * kp + 1)
            js = [jstart(kc, g) for kc in kcs]
            ws = [512 - 128 * j for j in js]
            cb = [0, ws[0]]
            ps_h0 = ps_mm.tile([128, 1024], f32, tag="mm")
            ps_h1 = ps_mm.tile([128, 1024], f32, tag="mm")
            pss = [ps_h0, ps_h1]
            for d in range(2):
                for hl in range(2):
                    nc.tensor.matmul(
                        pss[hl][:, cb[d]:cb[d] + ws[d]],
                        lhsT=kT_sb[hl * 64:(hl + 1) * 64, hp,
                                   kcs[d] * 128:(kcs[d] + 1) * 128],
                        rhs=qT_sb[hl * 64:(hl + 1) * 64, hp,
                                  g * 512 + js[d] * 128:(g + 1) * 512],
                        start=True, stop=True,
                    )
            for hl in range(2):
                et = epool.tile([128, 1024], bf16, tag="E")
                nc.scalar.activation(
                    et[:, 0:cb[1] + ws[1]], pss[hl][:, 0:cb[1] + ws[1]],
                    AF.Exp, scale=scale,
                )
                for d in range(2):
                    if kcs[d] >= c.QT * g:
                        nc.vector.tensor_tensor(
                            out=et[:, cb[d]:cb[d] + 128],
                            in0=et[:, cb[d]:cb[d] + 128],
                            in1=tri01[:], op=ALU.mult,
                        )
                boxes[hl].setdefault("ets", {})[kp] = (et, cb)
            for d in range(2):
                for hl in range(2):
                    feed_urgent.append(
                        (ws[d] / 2.4 + 10,
                         lambda hl=hl, kc=kcs[d], box=boxes[hl]:
                             pv_mm(hl, kc, box)))
            feed(2 * (cb[1] + ws[1] + 352) / 1.2 - ((ws[0] + ws[1]) / 2.4 + 40))

        # normalization split in two so the multiply never waits at the
        # head of the DVE queue for its broadcast DMA: part a (reciprocal
        # on the psum sums row + stride-0 broadcast) pops at group end,
        # part b (the multiply) pops a round later.
        def norm_a(box):
            # reciprocal of the sums row, then broadcast it down 64
            # partitions with a K=1 matmul (ones ⊗ row) + a copy to SBUF
            psv = box["psv"]
            rf = rfpool.tile([128, 512], f32, tag="rf")
            nc.vector.reciprocal_approx_fast(out=rf[0:1, :], in_=psv[0:1, :])
            rbp = ps_pv.tile([128, 512], f32, tag="pv")
            nc.tensor.matmul(rbp[64:128, :], lhsT=ones64[:], rhs=rf[0:1, :],
                             start=True, stop=True)
            rb = rbpool.tile([128, 512], f32, tag="rb")
            nc.vector.tensor_copy(rb[64:128, :], rbp[64:128, :])
            box["rb"] = rb

        def norm_b(hl, hp, g, box):
            psv, rb = box["psv"], box["rb"]
            if hl == 1:
                nc.vector.tensor_tensor(
                    out=aT_sb[64:128, hp, g * 512:(g + 1) * 512],
                    in0=psv[64:128, :], in1=rb[64:128, :], op=ALU.mult,
                )
            else:
                st = stg.tile([128, 512], bf16, tag="stg")
                nc.vector.tensor_tensor(
                    out=st[64:128, :], in0=psv[64:128, :],
                    in1=rb[64:128, :], op=ALU.mult,
                )
                nc.gpsimd.dma_start(
                    out=aT_sb[0:64, hp, g * 512:(g + 1) * 512],
                    in_=st[64:128, :],
                )

        for hl in range(2):
            feed_urgent.append((0, lambda box=boxes[hl]: norm_a(box)))
        for hl in range(2):
            feed_urgent.append(
                (0, lambda hl=hl, hp=hp, g=g, box=boxes[hl]:
                    norm_b(hl, hp, g, box)))

    ostage = ctx.enter_context(tc.tile_pool(name="ostage", bufs=4))

    def queue_outproj(g):
        for t in range(g * c.QT, (g + 1) * c.QT):
            def op_t(t=t):
                ps = ps_mm.tile([128, 1024], f32, tag="mm")
                for d in range(2):
                    for ci in range(c.MC):
                        nc.tensor.matmul(
                            ps[:, d * 512:(d + 1) * 512],
                            lhsT=aT_sb[:, ci, t * 128:(t + 1) * 128],
                            rhs=wo_sb[:, ci, d * 512:(d + 1) * 512],
                            start=(ci == 0), stop=(ci == c.MC - 1),
                        )
                ot = ostage.tile([128, 1024], bf16, tag="o")
                if t % 2 == 0:
                    nc.vector.tensor_copy(ot[:], ps[:])
                else:
                    nc.scalar.copy(ot[:], ps[:])
                nc.sync.dma_start(out=out[t * 128:(t + 1) * 128, :],
                                  in_=ot[:])
            feed_urgent.append((900, op_t))

    # ---- emission schedule ----
    # Prerequisite projection tiles are force-emitted just before the first
    # group whose scores/pv read them (emission order is program order for
    # the dependency tracker); pv/norm/outproj items are credit-fed between
    # score rounds so the PE never idles behind the exp backlog.
    emit_qk_tile(0, "q", 0)
    emit_qk_tile(0, "k", 0)
    # v tiles ride the urgent queue: each is queued ahead of the pv items
    # that read it, so it pops in time without delaying the first scores.
    feed_urgent.append((2000, lambda: emit_v_tile(0)))
    feed_urgent.append((2000, lambda: emit_v_tile(1)))

    qkt = emit_qk_tile
    queued_pre = {
        (0, 0): [(2000, lambda: emit_v_tile(2)),
                 (2000, lambda: emit_v_tile(3))],
        (0, 1): [(2000, lambda: emit_v_tile(4)),
                 (2000, lambda: emit_v_tile(5))],
        (0, 2): [(2000, lambda: emit_v_tile(6)),
                 (2000, lambda: emit_v_tile(7))],
    }
    # scores read qT/kT directly (not via the queue), so their projection
    # tiles must be force-emitted before the group that reads them; the
    # 512-wide quanta are spread to the groups with ACT headroom
    forced_pre = {
        (0, 1): [(0, emit_xT_h1),
                 (1700, lambda: qkt(0, "q", 1)),
                 (1700, lambda: qkt(0, "k", 1))],
        (0, 2): [(1700, lambda: qkt(0, "q", 2)),
                 (1700, lambda: qkt(0, "k", 2)),
                 (1700, lambda: qkt(1, "q", 0)),
                 (1700, lambda: qkt(1, "k", 0))],
        (0, 3): [(1700, lambda: qkt(0, "q", 3)),
                 (1700, lambda: qkt(0, "k", 3)),
                 (1700, lambda: qkt(1, "q", 1)),
                 (1700, lambda: qkt(1, "k", 1)),
                 (1700, lambda: qkt(1, "q", 2)),
                 (1700, lambda: qkt(1, "k", 2)),
                 (1700, lambda: qkt(1, "q", 3)),
                 (1700, lambda: qkt(1, "k", 3))],
    }

    for hp, gs in ((0, range(c.QG)), (1, reversed(range(c.QG)))):
        for g in gs:
            for cost, fn in forced_pre.get((hp, g), ()):
                force(cost, fn)
            attn_group(hp, g)
            for item in queued_pre.get((hp, g), ()):
                feed_urgent.append(item)
            if hp == 1:
                queue_outproj(g)
    flush_feed()

    if dbg:
        nc.sync.dma_start(out=dbg["qT"], in_=qT_sb[:])
        nc.sync.dma_start(out=dbg["kT"], in_=kT_sb[:])
        nc.sync.dma_start(out=dbg["v"], in_=v_sb[:])
        nc.sync.dma_start(out=dbg["aT"], in_=aT_sb[:])


# ---------------------------------------------------------------------------
# host side
# ---------------------------------------------------------------------------

_CACHE: dict = {}


def _get_program(cfg: Cfg):
    if cfg not in _CACHE:
        _CACHE[cfg] = build_program(cfg)
    return _CACHE[cfg]


def _mask_mode(mask: np.ndarray, T: int) -> str:
    m = (np.asarray(mask).reshape(T, T) != 0)
    if m.all():
        return "full"
    if np.array_equal(m, np.tril(np.ones((T, T), dtype=bool))):
        return "causal"
    return "bias"


def make_in_maps(cfg: Cfg, x, W_qkv, b_qkv, W_out, mask=None):
    c = cfg
    npmm = c.npmm
    B = x.shape[0]
    n_hg = N_CORES // B
    xTs = []
    for b in range(B):
        xa = np.zeros((c.DM + 128, c.T), dtype=npmm)
        xa[:c.DM] = np.ascontiguousarray(x[b].T).astype(npmm)
        xa[c.DM] = npmm(1.0)
        xTs.append(xa)
    in_maps = []
    for core in range(N_CORES):
        b, hg = divmod(core, n_hg)
        col0 = hg * c.NHD
        wq_ = np.ascontiguousarray(
            W_qkv[:, col0:col0 + c.NHD]).astype(npmm)
        wk_ = np.ascontiguousarray(
            W_qkv[:, c.DM + col0:c.DM + col0 + c.NHD]).astype(npmm)
        wv_ = np.zeros((c.DM + 128, c.NHD), dtype=npmm)
        wv_[:c.DM] = W_qkv[:, 2 * c.DM + col0:2 * c.DM + col0 + c.NHD].astype(npmm)
        wv_[c.DM] = b_qkv[2 * c.DM + col0:2 * c.DM + col0 + c.NHD].astype(npmm)
        bq_ = np.ascontiguousarray(
            b_qkv[col0:col0 + c.NHD].reshape(c.MC, 128).T).astype(np.float32)
        bk_ = np.ascontiguousarray(
            b_qkv[c.DM + col0:c.DM + col0 + c.NHD].reshape(c.MC, 128).T
        ).astype(np.float32)
        wo_ = np.ascontiguousarray(W_out[col0:col0 + c.NHD, :]).astype(npmm)
        in_maps.append(dict(xT=xTs[b], wq=wq_, wk=wk_, wv=wv_, bq=bq_,
                            bk=bk_, wo=wo_))
    return in_maps


def run_sharded(cfg: Cfg, x, W_qkv, b_qkv, W_out, b_out, mask=None, **kw):
    nc, _names = _get_program(cfg)
    in_maps = make_in_maps(cfg, x, W_qkv, b_qkv, W_out, mask)
    res = bass_utils.run_bass_kernel_spmd(
        nc, in_maps, core_ids=list(range(N_CORES)), **kw,
    )
    outs = [np.asarray(r["out"], dtype=np.float32) for r in res.results]
    B = x.shape[0]
    n_hg = N_CORES // B
    y = np.stack([
        np.sum(outs[b * n_hg:(b + 1) * n_hg], axis=0) for b in range(B)
    ]) + b_out.astype(np.float32)
    return y.astype(np.float32), res


def kernel(x, W_qkv, b_qkv, W_out, b_out, mask):
    x = np.asarray(x, dtype=np.float32)
    W_qkv = np.asarray(W_qkv, dtype=np.float32)
    b_qkv = np.asarray(b_qkv, dtype=np.float32)
    W_out = np.asarray(W_out, dtype=np.float32)
    b_out = np.asarray(b_out, dtype=np.float32)
    B, T, DM = x.shape
    mode = _mask_mode(mask, T)
    cfg = Cfg(T=T, DM=DM, mode=mode)
    y, _ = run_sharded(cfg, x, W_qkv, b_qkv, W_out, b_out, mask)
    return y


# revision 29
# speedup vs baseline: 1.1892x; 1.1892x over previous
"""Multi-head attention (B=2, T=2048, d_model=1024, H=16, hd=64) on 8 Trainium2
NeuronCores.

Sharding: the 32 (batch, head) attention units are split as 4 consecutive heads
of one batch per core (core c -> batch c//4, heads 4*(c%4) .. 4*(c%4)+3). Each
core computes its own QKV projection slice, causal attention for its heads, and
a partial out-projection (its 256 rows of W_out). The host sums the 4 partials
per batch and adds b_out.

Device-side layout (everything flows transposed so no on-chip transposes are
needed until the attention output):
  qT/kT [hd, T]  <- lhsT=W slice, rhs=xT
  v     [T, hd]  (+ ones column for the row-sum trick)
  sT    [k, q]   <- lhsT=kT chunk, rhs=qT          (psum, fp32)
  E     [k, q]   <- exp(sT * 1/sqrt(hd)) on ScalarE (bf16)
  pv    [q, hd+1]<- lhsT=E chunk, rhs=[v|1]        (col hd = row sum)
  a     [q, hd]  = pv[:, :hd] * (1/pv[:, hd])      (per-partition scalar)
  aT    [hd, T]  via DRAM round-trip DMA transpose
  out  += aT.T @ W_out slice                        (partial, fp32)
"""

import math
import os
from contextlib import ExitStack
from dataclasses import dataclass

import numpy as np
import ml_dtypes

import concourse.bass as bass
import concourse.tile as tile
from concourse import bacc, mybir
from concourse import bass_utils

AF = mybir.ActivationFunctionType
ALU = mybir.AluOpType
DT = mybir.dt

N_CORES = 8
NEG = -1e9


@dataclass(frozen=True)
class Cfg:
    T: int = 2048        # sequence length
    DM: int = 1024       # d_model
    HD: int = 64         # head dim
    NH: int = 4          # heads per core
    mode: str = "causal"  # "causal" | "full" | "bias"
    mm: str = "bf16"     # matmul operand dtype: "bf16" | "f32r" | "f32"

    @property
    def NHD(self):
        return self.NH * self.HD          # qkv slice width per core

    @property
    def KC(self):
        return self.DM // 128             # contraction chunks for projections

    @property
    def MC(self):
        return self.NHD // 128            # qT/kT partition chunks

    @property
    def TC(self):
        return self.T // 128              # t chunks

    @property
    def QW(self):
        return min(512, self.T)           # q group width

    @property
    def QG(self):
        return self.T // self.QW

    @property
    def QT(self):
        return self.QW // 128             # q tiles per group

    @property
    def EB(self):
        return self.DM // 512             # out-proj free blocks

    @property
    def mmdt(self):
        return {"bf16": DT.bfloat16, "f32r": DT.float32r, "f32": DT.float32}[self.mm]

    @property
    def npmm(self):
        return ml_dtypes.bfloat16 if self.mm == "bf16" else np.float32


def build_program(cfg: Cfg):
    """Build + compile the SPMD single-core program. Returns (nc, input_names)."""
    c = cfg
    assert c.DM % 128 == 0 and c.NHD % 128 == 0 and c.T % 512 == 0
    nc = bacc.Bacc("TRN2", target_bir_lowering=False, debug=False,
                   num_devices=N_CORES)
    f32 = DT.float32
    bf16 = DT.bfloat16
    mmdt = c.mmdt

    xT = nc.dram_tensor("xT", [c.DM, c.T], mmdt, kind="ExternalInput").ap()
    wq = nc.dram_tensor("wq", [c.DM, c.NHD], mmdt, kind="ExternalInput").ap()
    wk = nc.dram_tensor("wk", [c.DM, c.NHD], mmdt, kind="ExternalInput").ap()
    wv = nc.dram_tensor("wv", [c.DM, c.NHD], mmdt, kind="ExternalInput").ap()
    bq = nc.dram_tensor("bq", [128, c.MC], f32, kind="ExternalInput").ap()
    bk = nc.dram_tensor("bk", [128, c.MC], f32, kind="ExternalInput").ap()
    bvb = nc.dram_tensor("bvb", [128, c.NHD], f32, kind="ExternalInput").ap()
    wo = nc.dram_tensor("wo", [c.NHD, c.DM], mmdt, kind="ExternalInput").ap()
    maskb = None
    if c.mode == "bias":
        # additive bias, transposed: maskb[k, q]
        maskb = nc.dram_tensor("maskb", [c.T, c.T], f32, kind="ExternalInput").ap()
    out = nc.dram_tensor("out", [c.T, c.DM], f32, kind="ExternalOutput").ap()

    with tile.TileContext(nc) as tc, ExitStack() as ctx:
        _body(ctx, tc, c, xT, wq, wk, wv, bq, bk, bvb, wo, maskb, out)
    nc.compile()
    names = ["xT", "wq", "wk", "wv", "bq", "bk", "bvb", "wo"]
    if c.mode == "bias":
        names.append("maskb")
    return nc, names


def _body(ctx, tc, c: Cfg, xT, wq, wk, wv, bq, bk, bvb, wo, maskb, out):
    nc = tc.nc
    f32 = DT.float32
    bf16 = DT.bfloat16
    mmdt = c.mmdt
    causal = c.mode == "causal"
    scale = 1.0 / math.sqrt(c.HD)

    const = ctx.enter_context(tc.tile_pool(name="const", bufs=1))
    big = ctx.enter_context(tc.tile_pool(name="big", bufs=1))
    epool = ctx.enter_context(tc.tile_pool(name="E", bufs=c.TC))
    rpool = ctx.enter_context(tc.tile_pool(name="r", bufs=8))
    # PSUM: 3 x [128,1024] (6 banks) + 2 x [128,65] (2 banks) = 8 banks
    ps_mm = ctx.enter_context(tc.tile_pool(name="psmm", bufs=3, space="PSUM"))
    ps_pv = ctx.enter_context(tc.tile_pool(name="pspv", bufs=2, space="PSUM"))
    dramp = ctx.enter_context(tc.tile_pool(name="dram", bufs=1, space="DRAM"))
    bias_pool = None
    if c.mode == "bias":
        bias_pool = ctx.enter_context(tc.tile_pool(name="maskb", bufs=4))

    # ---- load inputs to SBUF ----
    bq_sb = const.tile([128, c.MC], f32, tag="bq")
    nc.sync.dma_start(out=bq_sb[:], in_=bq)
    bk_sb = const.tile([128, c.MC], f32, tag="bk")
    nc.sync.dma_start(out=bk_sb[:], in_=bk)
    bvb_sb = const.tile([128, c.NHD], f32, tag="bvb")
    nc.sync.dma_start(out=bvb_sb[:], in_=bvb)

    # consolidated input DMAs (one 3D-AP transfer each) — per-dma descriptor
    # generation on the sync sequencer is ~0.6us, so fewer, bigger DMAs
    # split along t so the first QKV block (which contracts over ALL chunks)
    # can start after the first half arrives
    xT_sb = big.tile([128, c.KC, c.T], mmdt, tag="xT")
    xTd = xT.rearrange("(c p) t -> p c t", p=128)
    TH = max(512, c.T // 2)
    for h in range(c.T // TH):
        nc.sync.dma_start(out=xT_sb[:, :, h * TH:(h + 1) * TH],
                          in_=xTd[:, :, h * TH:(h + 1) * TH])

    w_sbs = []
    for nm, w in (("wq", wq), ("wk", wk), ("wv", wv)):
        w_sb = big.tile([128, c.KC, c.NHD], mmdt, tag=nm)
        nc.sync.dma_start(out=w_sb[:],
                          in_=w.rearrange("(c p) n -> p c n", p=128))
        w_sbs.append(w_sb)
    wq_sb, wk_sb, wv_sb = w_sbs

    wo_sb = big.tile([128, c.MC, c.DM], mmdt, tag="wo")
    nc.sync.dma_start(out=wo_sb[:],
                      in_=wo.rearrange("(c p) n -> p c n", p=128))

    # causal mask block for diagonal tiles: tri[k, j] = 0 if j >= k else NEG
    tri = const.tile([128, 128], f32, tag="tri")
    nc.gpsimd.memset(tri[:], 0.0)
    nc.gpsimd.affine_select(
        out=tri[:], in_=tri[:],
        compare_op=ALU.is_ge, fill=NEG,
        base=0, channel_multiplier=-1, pattern=[[1, 128]],
    )

    # ---- QKV projections ----
    # psum tiles are [128, 1024] (2 banks); two 512-wide matmul groups per
    # tile, one wide DVE biased copy out.
    # qT is stored zero-padded per head ([128, NH, T], head h in partitions
    # (h%2)*64..+64, zeros elsewhere) so score matmuls can run with full
    # K=128 contraction: the other head's kT rows hit zeros. Full-K matmuls
    # keep the PE activity monitor busy -> 2.4 GHz instead of 1.2.
    qT_z = big.tile([128, c.NH, c.T], mmdt, tag="qT")
    nc.vector.memset(qT_z[:], 0.0)
    kT_sb = big.tile([128, c.MC, c.T], mmdt, tag="kT")
    HD1 = c.HD + 1
    v_sb = big.tile([128, c.TC, c.NH, HD1], bf16, tag="v")
    nc.vector.memset(v_sb[:, :, :, c.HD:HD1], 1.0)
    W2 = min(1024, c.T)
    VG = min(c.TC, max(1, 1024 // c.NHD))    # t-chunks per v psum tile

    def emit_qk_tile(m, w_sb, b_sb, which, n):
        ps = ps_mm.tile([128, 1024], f32, tag="mm")
        for d in range(W2 // 512):
            for k in range(c.KC):
                nc.tensor.matmul(
                    ps[:, d * 512:(d + 1) * 512],
                    lhsT=w_sb[:, k, m * 128:(m + 1) * 128],
                    rhs=xT_sb[:, k, n * W2 + d * 512:n * W2 + (d + 1) * 512],
                    start=(k == 0), stop=(k == c.KC - 1),
                )
        sl = slice(n * W2, (n + 1) * W2)
        if which == "k":
            nc.vector.tensor_scalar_add(
                kT_sb[:, m, sl], ps[:, 0:W2], b_sb[:, m:m + 1],
            )
        else:
            nc.vector.tensor_scalar_add(
                qT_z[0:64, 2 * m, sl], ps[0:64, 0:W2], b_sb[0:64, m:m + 1],
            )
            nc.vector.tensor_scalar_add(
                qT_z[64:128, 2 * m + 1, sl], ps[64:128, 0:W2],
                b_sb[64:128, m:m + 1],
            )

    def emit_qk(m):
        for w_sb, b_sb, which in ((wq_sb, bq_sb, "q"), (wk_sb, bk_sb, "k")):
            for n in range(c.T // W2):
                emit_qk_tile(m, w_sb, b_sb, which, n)

    def emit_v_tile(tg):
        # v in normal layout, augmented with a ones column per head;
        # VG t-chunks share one psum tile.
        ps = ps_mm.tile([128, 1024], f32, tag="mm")
        for d in range(VG):
            t = tg * VG + d
            for k in range(c.KC):
                nc.tensor.matmul(
                    ps[:, d * c.NHD:(d + 1) * c.NHD],
                    lhsT=xT_sb[:, k, t * 128:(t + 1) * 128],
                    rhs=wv_sb[:, k, :],
                    start=(k == 0), stop=(k == c.KC - 1),
                )
        for d in range(VG):
            t = tg * VG + d
            nc.vector.tensor_tensor(
                out=v_sb[:, t, :, 0:c.HD],
                in0=ps[:, d * c.NHD:(d + 1) * c.NHD].rearrange(
                    "p (h d) -> p h d", d=c.HD),
                in1=bvb_sb.rearrange("p (h d) -> p h d", d=c.HD),
                op=ALU.add,
            )

    # (emission of qk/v/attention is interleaved below: head-pair hp's
    # attention is emitted before chunk hp+1's q/k so the psum-slot FIFO
    # doesn't serialize attention behind the whole projection phase)

    # ---- attention (head-pair outer, q-group inner) ----
    # kc chunks are paired into [128, 1024] psum tiles so one exp covers
    # 1024 columns. Scores run with full K=128 contraction against the
    # natural two-head kT chunk (zero-padded qT kills the other head's
    # contribution), which keeps the PE activity monitor at 2.4 GHz.
    a_sb = big.tile([128, c.TC, c.NH, c.HD], bf16, tag="a")
    a_dram = dramp.tile([c.T, c.NHD], bf16, tag="adram")
    aT_sb = big.tile([128, c.MC, c.T], bf16, tag="aT")
    ostage = ctx.enter_context(tc.tile_pool(name="ostage", bufs=4))

    def attn_hp(hp, fillers=()):
        fillers = list(fillers)
        per_g = -(-len(fillers) // c.QG) if fillers else 0
        for g in range(c.QG):
            for _ in range(per_g):
                if fillers:
                    fillers.pop(0)()
            kmax = (g + 1) * c.QT if causal else c.TC
            assert kmax % 2 == 0
            etiles = {}                      # (hl, kp) -> [128, 1024] E tile
            for kp in range(kmax // 2):
                for hl in range(2):
                    h = 2 * hp + hl
                    ps = ps_mm.tile([128, 1024], f32, tag="mm")
                    for d in range(2):
                        kc = 2 * kp + d
                        nc.tensor.matmul(
                            ps[:, d * 512:d * 512 + c.QW],
                            lhsT=kT_sb[:, hp, kc * 128:(kc + 1) * 128],
                            rhs=qT_z[:, h, g * c.QW:(g + 1) * c.QW],
                            start=True, stop=True,
                        )
                        if causal:
                            off = (kc - g * c.QT) * 128
                            if off >= 0:
                                nc.vector.tensor_tensor(
                                    out=ps[:, d * 512 + off:d * 512 + off + 128],
                                    in0=ps[:, d * 512 + off:d * 512 + off + 128],
                                    in1=tri[:], op=ALU.add,
                                )
                        elif c.mode == "bias":
                            mb = bias_pool.tile([128, c.QW], f32, tag="mb")
                            nc.sync.dma_start(
                                out=mb[:],
                                in_=maskb[kc * 128:(kc + 1) * 128,
                                          g * c.QW:(g + 1) * c.QW],
                            )
                            nc.vector.tensor_tensor(
                                out=ps[:, d * 512:d * 512 + c.QW],
                                in0=ps[:, d * 512:d * 512 + c.QW],
                                in1=mb[:], op=ALU.add,
                            )
                    et = epool.tile([128, 1024], bf16, tag="E")
                    nc.scalar.activation(et[:], ps[:], AF.Exp, scale=scale)
                    etiles[(hl, kp)] = et
            for hl in range(2):
                h = 2 * hp + hl
                for j in range(c.QT):
                    qt = g * c.QT + j
                    kn = qt + 1 if causal else c.TC
                    psv = ps_pv.tile([128, HD1], f32, tag="pv")
                    for kc in range(kn):
                        kp, d = divmod(kc, 2)
                        nc.tensor.matmul(
                            psv[:],
                            lhsT=etiles[(hl, kp)][
                                :, d * 512 + j * 128:d * 512 + (j + 1) * 128],
                            rhs=v_sb[:, kc, h, :],
                            start=(kc == 0), stop=(kc == kn - 1),
                        )
                    r = rpool.tile([128, 1], f32, tag="r")
                    nc.vector.reciprocal(r[:], psv[:, c.HD:HD1])
                    nc.vector.tensor_scalar_mul(
                        a_sb[:, qt, h, :], psv[:, 0:c.HD], r[:, 0:1],
                    )
                    # stream a out to DRAM as soon as each t-chunk is done
                    if hp == c.NH // 2 - 1 and hl == 1:
                        nc.sync.dma_start(
                            out=a_dram[qt * 128:(qt + 1) * 128, :],
                            in_=a_sb[:, qt, :, :],
                        )

            # transpose this group's a rows -> aT as soon as they're final
            if hp == c.NH // 2 - 1:
                for ci in range(c.MC):
                    nc.sync.dma_start(
                        out=aT_sb[:, ci, g * c.QW:(g + 1) * c.QW],
                        in_=a_dram[g * c.QW:(g + 1) * c.QW,
                                   ci * 128:(ci + 1) * 128],
                        transpose=True,
                    )

    # Head pair hp needs only q/k chunk hp (+v). Emit pair hp+1's projection
    # tiles as fillers inside pair hp's attention groups so they overlap the
    # ACT-bound exp phase instead of serializing behind it in the psum FIFO.
    emit_qk(0)
    emit_v_tile(0)
    fillers = [lambda tg=tg: emit_v_tile(tg) for tg in range(1, c.TC // VG)]
    for w_sb, b_sb, which in ((wq_sb, bq_sb, "q"), (wk_sb, bk_sb, "k")):
        for n in range(c.T // W2):
            fillers.append(
                lambda w=w_sb, b=b_sb, wh=which, n=n: emit_qk_tile(1, w, b, wh, n))
    attn_hp(0, fillers)
    attn_hp(1)

    # ---- partial out-projection ----
    EW = min(1024, c.DM)
    for t in range(c.TC):
        for ebg in range(c.DM // EW):
            ps = ps_mm.tile([128, 1024], f32, tag="mm")
            for d in range(EW // 512):
                e0 = ebg * EW + d * 512
                for ci in range(c.MC):
                    nc.tensor.matmul(
                        ps[:, d * 512:(d + 1) * 512],
                        lhsT=aT_sb[:, ci, t * 128:(t + 1) * 128],
                        rhs=wo_sb[:, ci, e0:e0 + 512],
                        start=(ci == 0), stop=(ci == c.MC - 1),
                    )
            ot = ostage.tile([128, EW], f32, tag="o")
            if t % 2 == 0:
                nc.vector.tensor_copy(ot[:], ps[:, 0:EW])
            else:
                nc.scalar.copy(ot[:], ps[:, 0:EW])
            nc.sync.dma_start(
                out=out[t * 128:(t + 1) * 128, ebg * EW:(ebg + 1) * EW],
                in_=ot[:],
            )


# ---------------------------------------------------------------------------
# host side
# ---------------------------------------------------------------------------

_CACHE: dict = {}


def _get_program(cfg: Cfg):
    key = cfg
    if key not in _CACHE:
        _CACHE[key] = build_program(cfg)
    return _CACHE[key]


def _mask_mode(mask: np.ndarray, T: int) -> str:
    m = (np.asarray(mask).reshape(T, T) != 0)
    if m.all():
        return "full"
    if np.array_equal(m, np.tril(np.ones((T, T), dtype=bool))):
        return "causal"
    return "bias"


def make_in_maps(cfg: Cfg, x, W_qkv, b_qkv, W_out, mask=None):
    """Slice full inputs into the 8 per-core input dicts."""
    c = cfg
    npmm = c.npmm
    B = x.shape[0]
    n_hg = N_CORES // B                      # head groups per batch
    in_maps = []
    maskb = None
    if c.mode == "bias":
        m = (np.asarray(mask).reshape(c.T, c.T) != 0)
        maskb = np.where(m, np.float32(0), np.float32(NEG)).T.copy()
    for core in range(N_CORES):
        b, hg = divmod(core, n_hg)
        col0 = hg * c.NHD
        xT = np.ascontiguousarray(x[b].T).astype(npmm)
        wq_ = np.ascontiguousarray(W_qkv[:, 0 * c.DM + col0:0 * c.DM + col0 + c.NHD]).astype(npmm)
        wk_ = np.ascontiguousarray(W_qkv[:, 1 * c.DM + col0:1 * c.DM + col0 + c.NHD]).astype(npmm)
        wv_ = np.ascontiguousarray(W_qkv[:, 2 * c.DM + col0:2 * c.DM + col0 + c.NHD]).astype(npmm)
        bq_ = np.ascontiguousarray(
            b_qkv[0 * c.DM + col0:0 * c.DM + col0 + c.NHD].reshape(c.MC, 128).T
        ).astype(np.float32)
        bk_ = np.ascontiguousarray(
            b_qkv[1 * c.DM + col0:1 * c.DM + col0 + c.NHD].reshape(c.MC, 128).T
        ).astype(np.float32)
        bv_ = b_qkv[2 * c.DM + col0:2 * c.DM + col0 + c.NHD].astype(np.float32)
        bvb_ = np.ascontiguousarray(np.broadcast_to(bv_, (128, c.NHD)))
        wo_ = np.ascontiguousarray(W_out[col0:col0 + c.NHD, :]).astype(npmm)
        im = dict(xT=xT, wq=wq_, wk=wk_, wv=wv_, bq=bq_, bk=bk_, bvb=bvb_,
                  wo=wo_)
        if c.mode == "bias":
            im["maskb"] = maskb
        in_maps.append(im)
    return in_maps


def run_sharded(cfg: Cfg, x, W_qkv, b_qkv, W_out, b_out, mask=None, **kw):
    """Run the SPMD program on 8 cores and assemble the full output."""
    nc, _names = _get_program(cfg)
    in_maps = make_in_maps(cfg, x, W_qkv, b_qkv, W_out, mask)
    res = bass_utils.run_bass_kernel_spmd(
        nc, in_maps, core_ids=list(range(N_CORES)), **kw,
    )
    outs = [r["out"] for r in res.results]
    B = x.shape[0]
    n_hg = N_CORES // B
    y = np.stack([
        np.sum(outs[b * n_hg:(b + 1) * n_hg], axis=0) for b in range(B)
    ]) + b_out.astype(np.float32)
    return y.astype(np.float32), res


def kernel(x, W_qkv, b_qkv, W_out, b_out, mask):
    x = np.asarray(x, dtype=np.float32)
    W_qkv = np.asarray(W_qkv, dtype=np.float32)
    b_qkv = np.asarray(b_qkv, dtype=np.float32)
    W_out = np.asarray(W_out, dtype=np.float32)
    b_out = np.asarray(b_out, dtype=np.float32)
    B, T, DM = x.shape
    mode = _mask_mode(mask, T)
    cfg = Cfg(T=T, DM=DM, mode=mode, mm=os.environ.get("MHA_MM_DT", "bf16"))
    y, _ = run_sharded(cfg, x, W_qkv, b_qkv, W_out, b_out, mask)
    return y

